# revision 7
# baseline (speedup 1.0000x reference)
"""MultiHeadAttention Trainium2 kernel (8-core data-parallel over batch).

Per core (batch b):
  QT = wq @ q_b^T  [d_model, S]  (head h rows 64h..64h+64 = qh^T), KT likewise.
  V  = q-normal [S, d_model].
  N-orientation: scores[si,sj] (K=64) + rank-1 mask bias via K=1 ones matmul,
    ACT exp with fused row-sum accum -> E, A = E * (1/r) -> attn output.
  T-orientation: scoresT[sj,si], ACT exp with per-partition mask bias -> E_T,
    PV: OT[c,si] = sum_sj V-slices^T @ E_T (out at partition half by head
    parity), normalized by 1/r transposed to a row (PE transpose) and
    partition-broadcast.
  output = OUT_T-contract @ denseT + bias.
All matmul operands float32r (full PE rate at moving-dim 512).
"""

import numpy as np

import concourse.bass as bass
import concourse.tile as tile
from concourse import bacc, mybir
from concourse.bass_utils import run_bass_kernel_spmd
from concourse.masks import make_identity

F32R = mybir.dt.float32r
F32 = mybir.dt.float32
Exp = mybir.ActivationFunctionType.Exp
MULT = mybir.AluOpType.mult
ts = bass.ts
ds = bass.ds

B, S, D, H, C = 8, 1024, 1024, 16, 64
SCALE = float(1.0 / (np.sqrt(C) + 1e-8))
MASKNEG = -1e9

_CACHE = {}


def _build():
    nc = bacc.Bacc("TRN2", target_bir_lowering=False, debug=False)

    qT = nc.dram_tensor("qT", [D, S], F32R, kind="ExternalInput").ap()
    kT = nc.dram_tensor("kT", [D, S], F32R, kind="ExternalInput").ap()
    vT = nc.dram_tensor("vT", [D, S], F32R, kind="ExternalInput").ap()
    wqT = nc.dram_tensor("wqT", [D, D], F32R, kind="ExternalInput").ap()
    wkT = nc.dram_tensor("wkT", [D, D], F32R, kind="ExternalInput").ap()
    wvT = nc.dram_tensor("wvT", [D, D], F32R, kind="ExternalInput").ap()
    dT = nc.dram_tensor("dT", [D, D], F32R, kind="ExternalInput").ap()
    qb = nc.dram_tensor("qb", [128, 8], F32, kind="ExternalInput").ap()
    kb = nc.dram_tensor("kb", [128, 8], F32, kind="ExternalInput").ap()
    vb = nc.dram_tensor("vb", [1, D], F32R, kind="ExternalInput").ap()
    db = nc.dram_tensor("db", [1, D], F32R, kind="ExternalInput").ap()
    mneg = nc.dram_tensor("mneg", [1, S], F32R, kind="ExternalInput").ap()
    ones_in = nc.dram_tensor("ones", [1, 128], F32R, kind="ExternalInput").ap()
    mbias = nc.dram_tensor("mbias", [128, 8], F32, kind="ExternalInput").ap()

    out = nc.dram_tensor("out", [S, D], F32, kind="ExternalOutput").ap()
    attn = nc.dram_tensor("attn", [H, S, S], F32, kind="ExternalOutput").ap()

    with tile.TileContext(nc) as tc:
        with (
            tc.tile_pool(name="const", bufs=1) as cpool,
            tc.tile_pool(name="qt", bufs=8) as qt_pool,
            tc.tile_pool(name="kt", bufs=8) as kt_pool,
            tc.tile_pool(name="vv", bufs=8) as v_pool,
            tc.tile_pool(name="outt", bufs=8) as outt_pool,
        ):
            # ---- constants ----
            ones1 = cpool.tile([1, 128], F32R, tag="ones1")
            nc.sync.dma_start(ones1[:], ones_in[:, :])
            ident = cpool.tile([128, 128], F32, tag="ident")
            make_identity(nc, ident[:])
            mneg_t = cpool.tile([1, S], F32R, tag="mneg")
            nc.sync.dma_start(mneg_t[:], mneg[:, :])
            mbias_t = cpool.tile([128, 8], F32, tag="mbias")
            nc.sync.dma_start(mbias_t[:], mbias[:, :])
            qb_t = cpool.tile([128, 8], F32, tag="qb")
            nc.sync.dma_start(qb_t[:], qb[:, :])
            kb_t = cpool.tile([128, 8], F32, tag="kb")
            nc.sync.dma_start(kb_t[:], kb[:, :])

            QT = [qt_pool.tile([128, S], F32R, tag="qt", name="qt") for _ in range(8)]
            KT = [kt_pool.tile([128, S], F32R, tag="kt", name="kt") for _ in range(8)]
            V = [v_pool.tile([128, D], F32R, tag="vv", name="vv") for _ in range(8)]
            OUTT = [outt_pool.tile([128, S], F32R, tag="outt", name="outt") for _ in range(8)]

            # ---- phase 1: projections ----
            with (
                tc.tile_pool(name="xt", bufs=8) as xt_pool,
                tc.tile_pool(name="wt", bufs=8) as wt_pool,
                tc.tile_pool(name="vbp", bufs=1) as vb_pool,
                tc.tile_pool(name="psP", bufs=4, space="PSUM") as psP,
            ):
                vb_t = vb_pool.tile([1, D], F32R, tag="vb")
                nc.sync.dma_start(vb_t[:], vb[:, :])
                for x_dram, w_dram, kind in (
                    (qT, wqT, "q"),
                    (kT, wkT, "k"),
                    (vT, wvT, "v"),
                ):
                    xt = []
                    wt = []
                    for kk in range(8):
                        t = xt_pool.tile([128, S], F32R, tag="xt")
                        nc.sync.dma_start(t[:], x_dram[ts(kk, 128), :])
                        xt.append(t)
                        t = wt_pool.tile([128, D], F32R, tag="wt")
                        nc.sync.dma_start(t[:], w_dram[ts(kk, 128), :])
                        wt.append(t)
                    for m in range(8):
                        for n in range(2):
                            ps = psP.tile([128, 512], F32, tag="psP")
                            for kk in range(8):
                                if kind == "v":
                                    lhsT = xt[kk][:, ts(m, 128)]
                                    rhs = wt[kk][:, ts(n, 512)]
                                else:
                                    lhsT = wt[kk][:, ts(m, 128)]
                                    rhs = xt[kk][:, ts(n, 512)]
                                nc.tensor.matmul(
                                    ps[:], lhsT=lhsT, rhs=rhs,
                                    start=(kk == 0),
                                    stop=(kk == 7 and kind != "v"),
                                )
                            if kind == "q":
                                nc.vector.tensor_scalar_add(
                                    QT[m][:, ts(n, 512)], ps[:], qb_t[:, m:m + 1]
                                )
                            elif kind == "k":
                                nc.vector.tensor_scalar_add(
                                    KT[m][:, ts(n, 512)], ps[:], kb_t[:, m:m + 1]
                                )
                            else:
                                nc.tensor.matmul(
                                    ps[:], lhsT=ones1[0:1, :],
                                    rhs=vb_t[0:1, ts(n, 512)],
                                    start=False, stop=True,
                                )
                                nc.vector.tensor_copy(V[m][:, ts(n, 512)], ps[:])

            # ---- phase 2: attention ----
            with (
                tc.tile_pool(name="et", bufs=8) as et_pool,
                tc.tile_pool(name="en", bufs=2) as en_pool,
                tc.tile_pool(name="acc", bufs=16) as acc_pool,
                tc.tile_pool(name="rec", bufs=16) as rec_pool,
                tc.tile_pool(name="rrow", bufs=1) as rrow_pool,
                tc.tile_pool(name="pb", bufs=2) as pb_pool,
                tc.tile_pool(name="ps2", bufs=2, space="PSUM") as ps2_pool,
                tc.tile_pool(name="po", bufs=2, space="PSUM") as po_pool,
                tc.tile_pool(name="rp", bufs=1, space="PSUM") as rp_pool,
            ):
                for h in range(H):
                    t, off, parity = h // 2, (h % 2) * 64, h % 2
                    # --- N-orientation: attn output ---
                    recs = []
                    for m2 in range(8):
                        ps = ps2_pool.tile([128, 1024], F32, tag="ps2")
                        for n2 in range(2):
                            nc.tensor.matmul(
                                ps[:, ts(n2, 512)],
                                lhsT=QT[t][off:off + 64, ts(m2, 128)],
                                rhs=KT[t][off:off + 64, ts(n2, 512)],
                                start=True, stop=False,
                            )
                            nc.tensor.matmul(
                                ps[:, ts(n2, 512)],
                                lhsT=ones1[0:1, :],
                                rhs=mneg_t[0:1, ts(n2, 512)],
                                start=False, stop=True,
                            )
                        EN = en_pool.tile([128, 1024], F32, tag="en")
                        accv = acc_pool.tile([128, 1], F32, tag="acc")
                        nc.scalar.activation(
                            EN[:], ps[:], Exp, bias=0.0, scale=SCALE,
                            accum_out=accv[:],
                        )
                        rec = rec_pool.tile([128, 1], F32, tag="rec")
                        nc.vector.reciprocal(rec[:], accv[:])
                        recs.append(rec)
                        nc.vector.tensor_scalar_mul(EN[:], EN[:], rec[:])
                        nc.sync.dma_start(attn[h, ts(m2, 128), :], EN[:])
                    # --- T-orientation: E_T for PV ---
                    ET = [et_pool.tile([128, 1024], F32R, tag="et", name="et") for _ in range(8)]
                    for m in range(8):
                        ps = ps2_pool.tile([128, 1024], F32, tag="ps2")
                        for n in range(2):
                            nc.tensor.matmul(
                                ps[:, ts(n, 512)],
                                lhsT=KT[t][off:off + 64, ts(m, 128)],
                                rhs=QT[t][off:off + 64, ts(n, 512)],
                                start=True, stop=True,
                            )
                        nc.scalar.activation(
                            ET[m][:], ps[:], Exp,
                            bias=mbias_t[:, m:m + 1], scale=SCALE,
                        )
                    # --- 1/r row: PE-transpose the reciprocal columns ---
                    rrow_ps = rp_pool.tile([1, 1024], F32, tag="rp")
                    for m2 in range(8):
                        nc.tensor.transpose(
                            rrow_ps[0:1, ts(m2, 128)], recs[m2][:], ident[:]
                        )
                    rrow = rrow_pool.tile([1, 1024], F32, tag="rrow")
                    nc.vector.tensor_copy(rrow[:], rrow_ps[:])
                    # --- PV + normalize ---
                    for n in range(2):
                        po = po_pool.tile([128, 512], F32, tag="po")
                        # lhsT = the full head-pair block of V: rows off..off+64
                        # of the product are this head's OT; the other half is
                        # the sibling head's columns against this head's E_T
                        # (computed and ignored -- same N-cycle cost, and keeps
                        # the fp32r dst-partition-0 requirement satisfied).
                        ot_ap = po[off:off + 64, :]
                        for m in range(8):
                            nc.tensor.matmul(
                                po[:],
                                lhsT=V[m][:, ts(t, 128)],
                                rhs=ET[m][:, ts(n, 512)],
                                start=(m == 0), stop=(m == 7),
                            )
                        pb = pb_pool.tile([128, 512], F32, tag="pb")
                        pb_slice = pb[0:64, :] if parity == 0 else pb[64:128, :]
                        nc.gpsimd.partition_broadcast(
                            pb[:, :], rrow[0:1, ts(n, 512)], channels=128
                        )
                        nc.vector.tensor_tensor(
                            OUTT[t][off:off + 64, ts(n, 512)], ot_ap, pb_slice, MULT
                        )

            # ---- phase 3: dense ----
            with (
                tc.tile_pool(name="dt", bufs=8) as dt_pool,
                tc.tile_pool(name="dbp", bufs=1) as db_pool,
                tc.tile_pool(name="psD", bufs=4, space="PSUM") as psD,
                tc.tile_pool(name="osb", bufs=3) as osb_pool,
            ):
                db_t = db_pool.tile([1, D], F32R, tag="db")
                nc.sync.dma_start(db_t[:], db[:, :])
                dt = []
                for kk in range(8):
                    tt = dt_pool.tile([128, D], F32R, tag="dt")
                    nc.sync.dma_start(tt[:], dT[ts(kk, 128), :])
                    dt.append(tt)
                for m in range(8):
                    osb = osb_pool.tile([128, D], F32, tag="osb")
                    for n in range(2):
                        ps = psD.tile([128, 512], F32, tag="psD")
                        for kk in range(8):
                            nc.tensor.matmul(
                                ps[:], lhsT=OUTT[kk][:, ts(m, 128)],
                                rhs=dt[kk][:, ts(n, 512)],
                                start=(kk == 0), stop=False,
                            )
                        nc.tensor.matmul(
                            ps[:], lhsT=ones1[0:1, :], rhs=db_t[0:1, ts(n, 512)],
                            start=False, stop=True,
                        )
                        nc.vector.tensor_copy(osb[:, ts(n, 512)], ps[:])
                    nc.sync.dma_start(out[ts(m, 128), :], osb[:])

    nc.compile()
    return nc


def get_nc():
    if "nc" not in _CACHE:
        _CACHE["nc"] = _build()
    return _CACHE["nc"]


def make_in_maps(q, k, v, mask, wq_w, wq_b, wk_w, wk_b, wv_w, wv_b, dense_w, dense_b):
    q = np.asarray(q, np.float32)
    k = np.asarray(k, np.float32)
    v = np.asarray(v, np.float32)
    mask = np.asarray(mask)
    shared = {
        "wqT": np.ascontiguousarray(np.asarray(wq_w, np.float32).T),
        "wkT": np.ascontiguousarray(np.asarray(wk_w, np.float32).T),
        "wvT": np.ascontiguousarray(np.asarray(wv_w, np.float32).T),
        "dT": np.ascontiguousarray(np.asarray(dense_w, np.float32).T),
        "qb": np.ascontiguousarray(np.asarray(wq_b, np.float32).reshape(8, 128).T),
        "kb": np.ascontiguousarray(np.asarray(wk_b, np.float32).reshape(8, 128).T),
        "vb": np.asarray(wv_b, np.float32).reshape(1, D).copy(),
        "db": np.asarray(dense_b, np.float32).reshape(1, D).copy(),
    }
    shared["ones"] = np.ones((1, 128), np.float32)
    in_maps = []
    for b in range(B):
        mneg = np.zeros((1, S), np.float32)
        mneg[0, 1:] = np.where(np.asarray(mask[b, 0, 0]) == 0, MASKNEG, 0.0)
        mb = np.ascontiguousarray(mneg.reshape(8, 128).T)
        in_maps.append({
            **shared,
            "qT": np.ascontiguousarray(q[b].T),
            "kT": np.ascontiguousarray(k[b].T),
            "vT": np.ascontiguousarray(v[b].T),
            "mneg": mneg,
            "mbias": mb,
        })
    return in_maps


def kernel(**inputs):
    nc = get_nc()
    in_maps = make_in_maps(**inputs)
    res = run_bass_kernel_spmd(nc, in_maps, core_ids=list(range(B)))
    out = np.stack([res.results[b]["out"] for b in range(B)])
    attn = np.stack([res.results[b]["attn"] for b in range(B)])
    return out, attn
